# revision 19
# baseline (speedup 1.0000x reference)
"""Trainium2 Bass kernel for visual cross-attention:
    proj   = text @ W_w.T + W_b          [B,T,D]
    scores = proj @ local.T              [B,T,L]
    attn   = softmax(scores, axis=-1)
    out    = attn @ local                [B,T,D]

B=16, T=L=D=1024, fp32. Data-parallel over batch: 8 cores x 2 batches.
All matmuls run as float32r (full PE rate, ~1.5e-4 rel err vs 4x-slower fp32).

Per core, per batch, per T-tile (512 t's):
  A: projT[e,t]   = W^T-chunks.T @ textT-chunks        (PE, accum over d)
  B: scores[t,l]  = projT-chunks.T @ localT-chunks     (PE, accum over e)
     softmax over l (free dim): DVE max, ACT exp(+bias,-max, accum sum),
     DVE reciprocal + row scale
  T: attnT[l,t]   = PE transpose of attn[t,l] 128x128 blocks
     (emitted one q-chunk behind B so the next B covers softmax latency)
  C: outT[d,t]    = local-chunks.T @ attnT-chunks      (PE, accum over l)
For batch 0 both T-tiles' A phases are emitted first: the startup is
DMA-bound (~14MB must land) and A is the only work whose data arrives first.
Host side only reshapes/transposes (layout prep + final [d,t]->[t,d]).
"""
import sys

sys.path.insert(0, "/opt/trn_rl_repo")
import numpy as np

B, T, L, D = 16, 1024, 1024, 1024
NCORES = 8
NB = B // NCORES          # batches per core
TT = 512                  # T-tile (moving dim for phases A/C)
NT = T // TT              # T-tiles per batch
NC8 = D // 128            # 128-chunks along d/e/l
NQ = TT // 128            # 128-t chunks per T-tile

_cache = {}


def _build():
    import concourse.tile as tile
    from concourse import bacc, mybir
    from concourse.masks import make_identity

    f32 = mybir.dt.float32
    f32r = mybir.dt.float32r
    Act = mybir.ActivationFunctionType

    nc = bacc.Bacc("TRN2", target_bir_lowering=False, debug=False,
                   num_devices=NCORES)
    tT_d = nc.dram_tensor("tT", [NB, D, T], f32r, kind="ExternalInput").ap()
    lT_d = nc.dram_tensor("lT", [NB, D, L], f32r, kind="ExternalInput").ap()
    lN_d = nc.dram_tensor("lN", [NB, L, D], f32r, kind="ExternalInput").ap()
    wT_d = nc.dram_tensor("wT", [D, D], f32r, kind="ExternalInput").ap()
    wb_d = nc.dram_tensor("wb", [128, NC8], f32, kind="ExternalInput").ap()
    outT_d = nc.dram_tensor("outT", [NB, D, T], f32, kind="ExternalOutput").ap()

    with tile.TileContext(nc) as tc:
        with tc.tile_pool(name="const", bufs=1) as constp, \
             tc.tile_pool(name="res", bufs=1) as resp, \
             tc.tile_pool(name="work", bufs=2) as workp, \
             tc.tile_pool(name="proj", bufs=3) as projp, \
             tc.tile_pool(name="single", bufs=1) as singlep, \
             tc.tile_pool(name="stats", bufs=8) as statsp, \
             tc.tile_pool(name="psS", bufs=2, space="PSUM") as psS_p, \
             tc.tile_pool(name="psMM", bufs=2, space="PSUM") as psMM_p, \
             tc.tile_pool(name="psT", bufs=2, space="PSUM") as psT_p:

            # round-robin loads across all 3 DMA-capable queues (sync/scalar
            # HWDGE + gpsimd SWDGE), in consumption order; each queue peaks
            # ~110-130GB/s, together ~350GB/s (HBM-bound).
            queues = [nc.sync, nc.scalar, nc.gpsimd]
            qi = [0]

            def load(out, in_):
                queues[qi[0] % 3].dma_start(out=out, in_=in_)
                qi[0] += 1

            def load_tT(b, it):
                t0 = it * TT
                tile_ = workp.tile([128, NC8, TT], f32r, tag="tT")
                for dc in range(NC8):
                    load(tile_[:, dc, :],
                         tT_d[b, dc * 128:(dc + 1) * 128, t0:t0 + TT])
                return tile_

            wt_sb = constp.tile([128, NC8, D], f32r, tag="wt")
            wb_sb = constp.tile([128, NC8], f32, tag="wb")
            tT_first = workp.tile([128, NC8, TT], f32r, tag="tT")
            for dc in range(NC8):
                load(wt_sb[:, dc, :], wT_d[dc * 128:(dc + 1) * 128, :])
                load(tT_first[:, dc, :], tT_d[0, dc * 128:(dc + 1) * 128, 0:TT])
            load(wb_sb[:], wb_d[:])
            identf = constp.tile([128, 128], f32, tag="identf")
            make_identity(nc, identf[:])
            ident = constp.tile([128, 128], f32r, tag="ident")
            nc.vector.tensor_copy(ident[:], identf[:])

            def phase_a(tT_sb):
                projT = projp.tile([128, NC8, TT], f32r, tag="projT")
                for ec in range(NC8):
                    psA = psMM_p.tile([128, TT], f32, tag="mm")
                    for dc in range(NC8):
                        nc.tensor.matmul(
                            psA[:],
                            wt_sb[:, dc, ec * 128:(ec + 1) * 128],
                            tT_sb[:, dc, :],
                            start=(dc == 0), stop=(dc == NC8 - 1))
                    nc.scalar.activation(projT[:, ec, :], psA[:], Act.Identity,
                                         bias=wb_sb[:, ec:ec + 1], scale=1.0)
                return projT

            def transposes(attnT, et, q):
                for lq in range(NC8):
                    psT = psT_p.tile([128, 128], f32r, tag="tp")
                    nc.tensor.transpose(psT[:], et[:, lq * 128:(lq + 1) * 128],
                                        ident[:])
                    dst = attnT[:, lq, q * 128:(q + 1) * 128]
                    if lq % 2 == 0:
                        nc.vector.tensor_copy(dst, psT[:])
                    else:
                        nc.scalar.copy(dst, psT[:])

            # ---- batch-0 startup choreography ----
            # The first exp needs wt+tT(0,0)+tT(0,1)+tT(1,0)+lT(0) = 14MB of
            # HBM; PE bridge work (3 A phases + B-lo) is emitted first, in
            # the DMA arrival order, so the PE never goes HAM-cold.
            tT_b00 = tT_first

            tT_b01 = load_tT(0, 1)
            # borrow the (idle until ~48us) attnT slot for batch 1's first
            # text tile — a third live tT buffer doesn't fit in SBUF
            tT_b10 = singlep.tile([128, NC8, TT], f32r, tag="attnT")
            for dc in range(NC8):
                load(tT_b10[:, dc, :], tT_d[1, dc * 128:(dc + 1) * 128, 0:TT])
            lT_b0 = resp.tile([128, NC8, L], f32r, tag="lT")
            lN_b0 = resp.tile([128, NC8, D], f32r, tag="lN")
            for c in range(NC8):
                load(lT_b0[:, c, 0:512], lT_d[0, c * 128:(c + 1) * 128, 0:512])
            for c in range(NC8):
                load(lT_b0[:, c, 512:L], lT_d[0, c * 128:(c + 1) * 128, 512:L])
            for c in range(NC8):
                load(lN_b0[:, c, :], lN_d[0, c * 128:(c + 1) * 128, :])
            pre_proj = {(0, 0): phase_a(tT_b00), (0, 1): phase_a(tT_b01),
                        (1, 0): phase_a(tT_b10)}

            for b in range(NB):
                if b == 0:
                    lT_sb, lN_sb = lT_b0, lN_b0
                else:
                    lT_sb = resp.tile([128, NC8, L], f32r, tag="lT")
                    lN_sb = resp.tile([128, NC8, D], f32r, tag="lN")
                    for c in range(NC8):
                        load(lT_sb[:, c, :], lT_d[b, c * 128:(c + 1) * 128, :])
                    for c in range(NC8):
                        load(lN_sb[:, c, :], lN_d[b, c * 128:(c + 1) * 128, :])

                projTs = [pre_proj.get((b, it)) for it in range(NT)]
                tT_tiles = [None] * NT

                for it in range(NT):
                    t0 = it * TT
                    if projTs[it] is None:
                        if tT_tiles[it] is None:
                            tT_tiles[it] = load_tT(b, it)
                        projTs[it] = phase_a(tT_tiles[it])
                    projT = projTs[it]
                    # ---- phase B + softmax, transposes one q behind ----
                    attnT = singlep.tile([128, NC8, TT], f32r, tag="attnT")
                    pending = None
                    for q in range(NQ):
                        psS = psS_p.tile([128, L], f32, tag="scores")
                        for lh in range(L // 512):
                            for ec in range(NC8):
                                nc.tensor.matmul(
                                    psS[:, lh * 512:(lh + 1) * 512],
                                    projT[:, ec, q * 128:(q + 1) * 128],
                                    lT_sb[:, ec, lh * 512:(lh + 1) * 512],
                                    start=(ec == 0), stop=(ec == NC8 - 1))
                        nm = statsp.tile([128, 1], f32, tag="nm")
                        nc.vector.tensor_reduce(nm[:], psS[:],
                                                axis=mybir.AxisListType.X,
                                                op=mybir.AluOpType.max,
                                                negate=True)
                        et = workp.tile([128, L], f32r, tag="et")
                        s = statsp.tile([128, 1], f32, tag="s")
                        nc.scalar.activation(et[:], psS[:], Act.Exp,
                                             bias=nm[:, 0:1], scale=1.0,
                                             accum_out=s[:])
                        rr = statsp.tile([128, 1], f32, tag="rr")
                        nc.vector.reciprocal(rr[:], s[:])
                        nc.vector.tensor_scalar_mul(et[:], et[:], rr[:, 0:1])
                        if pending is not None:
                            transposes(attnT, *pending)
                        pending = (et, q)
                    transposes(attnT, *pending)
                    # ---- phase C: outT[d, t] ----
                    for dc in range(NC8):
                        psC = psMM_p.tile([128, TT], f32, tag="mm")
                        for lq in range(NC8):
                            nc.tensor.matmul(
                                psC[:],
                                lN_sb[:, lq, dc * 128:(dc + 1) * 128],
                                attnT[:, lq, :],
                                start=(lq == 0), stop=(lq == NC8 - 1))
                        outcp = workp.tile([128, TT], f32, tag="outcp")
                        nc.scalar.copy(outcp[:], psC[:])
                        queues[dc % 3].dma_start(
                            out=outT_d[b, dc * 128:(dc + 1) * 128, t0:t0 + TT],
                            in_=outcp[:])
    nc.compile()
    return nc


def _get_nc():
    if "nc" not in _cache:
        _cache["nc"] = _build()
    return _cache["nc"]


def _prep_inputs(text_features, local_features, W_w, W_b):
    text = np.asarray(text_features, dtype=np.float32)
    local = np.asarray(local_features, dtype=np.float32)
    W = np.asarray(W_w, dtype=np.float32)
    bvec = np.asarray(W_b, dtype=np.float32)

    wT = np.ascontiguousarray(W.T)                       # [d, e]
    wb = np.ascontiguousarray(bvec.reshape(NC8, 128).T)  # [128, ec]
    in_maps = []
    for c in range(NCORES):
        sl = slice(c * NB, (c + 1) * NB)
        in_maps.append({
            "tT": np.ascontiguousarray(text[sl].transpose(0, 2, 1)),
            "lT": np.ascontiguousarray(local[sl].transpose(0, 2, 1)),
            "lN": np.ascontiguousarray(local[sl]),
            "wT": wT,
            "wb": wb,
        })
    return in_maps


def _run(inputs, trace=False):
    from concourse.bass_utils import run_bass_kernel_spmd

    nc = _get_nc()
    in_maps = _prep_inputs(**inputs)
    res = run_bass_kernel_spmd(nc, in_maps, list(range(NCORES)), trace=trace)
    out = np.empty((B, T, D), dtype=np.float32)
    for c in range(NCORES):
        outT = res.results[c]["outT"]                    # [NB, d, t]
        out[c * NB:(c + 1) * NB] = outT.transpose(0, 2, 1)
    return out, res


def kernel(**inputs):
    out, _ = _run(inputs, trace=False)
    return out


# revision 21
# speedup vs baseline: 1.0173x; 1.0173x over previous
"""Trainium2 Bass kernel for visual cross-attention:
    proj   = text @ W_w.T + W_b          [B,T,D]
    scores = proj @ local.T              [B,T,L]
    attn   = softmax(scores, axis=-1)
    out    = attn @ local                [B,T,D]

B=16, T=L=D=1024, fp32. Data-parallel over batch: 8 cores x 2 batches.
All matmuls run as float32r (full PE rate, ~1.5e-4 rel err vs 4x-slower fp32).

Per core, per batch, per T-tile (512 t's):
  A: projT[e,t]   = W^T-chunks.T @ textT-chunks        (PE, accum over d)
  B: scores[t,l]  = projT-chunks.T @ localT-chunks     (PE, accum over e)
     softmax over l (free dim): DVE max, ACT exp(+bias,-max, accum sum),
     DVE reciprocal + row scale
  T: attnT[l,t]   = PE transpose of attn[t,l] 128x128 blocks
     (emitted one q-chunk behind B so the next B covers softmax latency)
  C: outT[d,t]    = local-chunks.T @ attnT-chunks      (PE, accum over l)
For batch 0 both T-tiles' A phases are emitted first: the startup is
DMA-bound (~14MB must land) and A is the only work whose data arrives first.
Host side only reshapes/transposes (layout prep + final [d,t]->[t,d]).
"""
import sys

sys.path.insert(0, "/opt/trn_rl_repo")
import numpy as np

B, T, L, D = 16, 1024, 1024, 1024
NCORES = 8
NB = B // NCORES          # batches per core
TT = 512                  # T-tile (moving dim for phases A/C)
NT = T // TT              # T-tiles per batch
NC8 = D // 128            # 128-chunks along d/e/l
NQ = TT // 128            # 128-t chunks per T-tile

_cache = {}


def _build():
    import concourse.tile as tile
    from concourse import bacc, mybir
    from concourse.masks import make_identity

    f32 = mybir.dt.float32
    f32r = mybir.dt.float32r
    Act = mybir.ActivationFunctionType

    nc = bacc.Bacc("TRN2", target_bir_lowering=False, debug=False,
                   num_devices=NCORES)
    tT_d = nc.dram_tensor("tT", [NB, D, T], f32r, kind="ExternalInput").ap()
    lT_d = nc.dram_tensor("lT", [NB, D, L], f32r, kind="ExternalInput").ap()
    lN_d = nc.dram_tensor("lN", [NB, L, D], f32r, kind="ExternalInput").ap()
    wT_d = nc.dram_tensor("wT", [D, D], f32r, kind="ExternalInput").ap()
    wb_d = nc.dram_tensor("wb", [128, NC8], f32, kind="ExternalInput").ap()
    outT_d = nc.dram_tensor("outT", [NB, D, T], f32, kind="ExternalOutput").ap()

    with tile.TileContext(nc) as tc:
        with tc.tile_pool(name="const", bufs=1) as constp, \
             tc.tile_pool(name="res", bufs=1) as resp, \
             tc.tile_pool(name="work", bufs=2) as workp, \
             tc.tile_pool(name="proj", bufs=2) as projp, \
             tc.tile_pool(name="single", bufs=1) as singlep, \
             tc.tile_pool(name="stats", bufs=8) as statsp, \
             tc.tile_pool(name="psS", bufs=2, space="PSUM") as psS_p, \
             tc.tile_pool(name="psMM", bufs=2, space="PSUM") as psMM_p, \
             tc.tile_pool(name="psT", bufs=2, space="PSUM") as psT_p:

            # round-robin loads across all 3 DMA-capable queues (sync/scalar
            # HWDGE + gpsimd SWDGE), in consumption order; each queue peaks
            # ~110-130GB/s, together ~350GB/s (HBM-bound).
            queues = [nc.sync, nc.scalar, nc.gpsimd]
            qi = [0]

            def load(out, in_):
                queues[qi[0] % 3].dma_start(out=out, in_=in_)
                qi[0] += 1

            def load_tT(b, it):
                t0 = it * TT
                tile_ = workp.tile([128, NC8, TT], f32r, tag="tT")
                for dc in range(NC8):
                    load(tile_[:, dc, :],
                         tT_d[b, dc * 128:(dc + 1) * 128, t0:t0 + TT])
                return tile_

            wt_sb = constp.tile([128, NC8, D], f32r, tag="wt")
            wb_sb = constp.tile([128, NC8], f32, tag="wb")
            tT_first = workp.tile([128, NC8, TT], f32r, tag="tT")
            for dc in range(NC8):
                load(wt_sb[:, dc, :], wT_d[dc * 128:(dc + 1) * 128, :])
                load(tT_first[:, dc, :], tT_d[0, dc * 128:(dc + 1) * 128, 0:TT])
            load(wb_sb[:], wb_d[:])
            identf = constp.tile([128, 128], f32, tag="identf")
            make_identity(nc, identf[:])
            ident = constp.tile([128, 128], f32r, tag="ident")
            nc.vector.tensor_copy(ident[:], identf[:])

            scr_f = constp.tile([128, TT], f32, tag="scr_f")
            nc.vector.memset(scr_f[:], 0.0)
            scr_w = constp.tile([128, 128], f32r, tag="scr_w")
            scr_r = constp.tile([128, TT], f32r, tag="scr_r")
            nc.vector.tensor_copy(scr_w[:], scr_f[:, 0:128])
            nc.vector.tensor_copy(scr_r[:], scr_f[:])
            for _ in range(12):
                ps = psMM_p.tile([128, TT], f32, tag="mm")
                nc.tensor.matmul(ps[:], scr_w[:], scr_r[:],
                                 start=True, stop=True)

            def phase_a(tT_sb):
                projT = projp.tile([128, NC8, TT], f32r, tag="projT")
                for ec in range(NC8):
                    psA = psMM_p.tile([128, TT], f32, tag="mm")
                    for dc in range(NC8):
                        nc.tensor.matmul(
                            psA[:],
                            wt_sb[:, dc, ec * 128:(ec + 1) * 128],
                            tT_sb[:, dc, :],
                            start=(dc == 0), stop=(dc == NC8 - 1))
                    nc.scalar.activation(projT[:, ec, :], psA[:], Act.Identity,
                                         bias=wb_sb[:, ec:ec + 1], scale=1.0)
                return projT

            def transposes(attnT, et, q):
                for lq in range(NC8):
                    psT = psT_p.tile([128, 128], f32r, tag="tp")
                    nc.tensor.transpose(psT[:], et[:, lq * 128:(lq + 1) * 128],
                                        ident[:])
                    dst = attnT[:, lq, q * 128:(q + 1) * 128]
                    if lq % 2 == 0:
                        nc.vector.tensor_copy(dst, psT[:])
                    else:
                        nc.scalar.copy(dst, psT[:])

            # ---- batch-0 startup choreography ----
            # The first exp needs wt+tT(0,0)+tT(0,1)+tT(1,0)+lT(0) = 14MB of
            # HBM; PE bridge work (3 A phases + B-lo) is emitted first, in
            # the DMA arrival order, so the PE never goes HAM-cold.
            tT_b00 = tT_first

            lT_b0 = resp.tile([128, NC8, L], f32r, tag="lT")
            lN_b0 = resp.tile([128, NC8, D], f32r, tag="lN")
            for c in range(NC8):
                load(lT_b0[:, c, 0:512], lT_d[0, c * 128:(c + 1) * 128, 0:512])
            tT_b01 = load_tT(0, 1)
            for c in range(NC8):
                load(lT_b0[:, c, 512:L], lT_d[0, c * 128:(c + 1) * 128, 512:L])
            for c in range(NC8):
                load(lN_b0[:, c, :], lN_d[0, c * 128:(c + 1) * 128, :])
            pre_proj = {(0, 0): phase_a(tT_b00), (0, 1): phase_a(tT_b01)}

            for b in range(NB):
                if b == 0:
                    lT_sb, lN_sb = lT_b0, lN_b0
                else:
                    lT_sb = resp.tile([128, NC8, L], f32r, tag="lT")
                    lN_sb = resp.tile([128, NC8, D], f32r, tag="lN")
                    for c in range(NC8):
                        load(lT_sb[:, c, :], lT_d[b, c * 128:(c + 1) * 128, :])
                    for c in range(NC8):
                        load(lN_sb[:, c, :], lN_d[b, c * 128:(c + 1) * 128, :])

                projTs = [pre_proj.get((b, it)) for it in range(NT)]
                tT_tiles = [None] * NT

                for it in range(NT):
                    t0 = it * TT
                    if projTs[it] is None:
                        if tT_tiles[it] is None:
                            tT_tiles[it] = load_tT(b, it)
                        projTs[it] = phase_a(tT_tiles[it])
                    projT = projTs[it]
                    # ---- phase B + softmax, transposes one q behind ----
                    attnT = singlep.tile([128, NC8, TT], f32r, tag="attnT")
                    pending = None
                    for q in range(NQ):
                        psS = psS_p.tile([128, L], f32, tag="scores")
                        for lh in range(L // 512):
                            for ec in range(NC8):
                                nc.tensor.matmul(
                                    psS[:, lh * 512:(lh + 1) * 512],
                                    projT[:, ec, q * 128:(q + 1) * 128],
                                    lT_sb[:, ec, lh * 512:(lh + 1) * 512],
                                    start=(ec == 0), stop=(ec == NC8 - 1))
                        nm = statsp.tile([128, 1], f32, tag="nm")
                        nc.vector.tensor_reduce(nm[:], psS[:],
                                                axis=mybir.AxisListType.X,
                                                op=mybir.AluOpType.max,
                                                negate=True)
                        et = workp.tile([128, L], f32r, tag="et")
                        s = statsp.tile([128, 1], f32, tag="s")
                        nc.scalar.activation(et[:], psS[:], Act.Exp,
                                             bias=nm[:, 0:1], scale=1.0,
                                             accum_out=s[:])
                        rr = statsp.tile([128, 1], f32, tag="rr")
                        nc.vector.reciprocal(rr[:], s[:])
                        nc.vector.tensor_scalar_mul(et[:], et[:], rr[:, 0:1])
                        if pending is not None:
                            transposes(attnT, *pending)
                        pending = (et, q)
                    transposes(attnT, *pending)
                    # ---- phase C: outT[d, t] ----
                    for dc in range(NC8):
                        psC = psMM_p.tile([128, TT], f32, tag="mm")
                        for lq in range(NC8):
                            nc.tensor.matmul(
                                psC[:],
                                lN_sb[:, lq, dc * 128:(dc + 1) * 128],
                                attnT[:, lq, :],
                                start=(lq == 0), stop=(lq == NC8 - 1))
                        outcp = workp.tile([128, TT], f32, tag="outcp")
                        nc.scalar.copy(outcp[:], psC[:])
                        queues[dc % 3].dma_start(
                            out=outT_d[b, dc * 128:(dc + 1) * 128, t0:t0 + TT],
                            in_=outcp[:])
    nc.compile()
    return nc


def _get_nc():
    if "nc" not in _cache:
        _cache["nc"] = _build()
    return _cache["nc"]


def _prep_inputs(text_features, local_features, W_w, W_b):
    text = np.asarray(text_features, dtype=np.float32)
    local = np.asarray(local_features, dtype=np.float32)
    W = np.asarray(W_w, dtype=np.float32)
    bvec = np.asarray(W_b, dtype=np.float32)

    wT = np.ascontiguousarray(W.T)                       # [d, e]
    wb = np.ascontiguousarray(bvec.reshape(NC8, 128).T)  # [128, ec]
    in_maps = []
    for c in range(NCORES):
        sl = slice(c * NB, (c + 1) * NB)
        in_maps.append({
            "tT": np.ascontiguousarray(text[sl].transpose(0, 2, 1)),
            "lT": np.ascontiguousarray(local[sl].transpose(0, 2, 1)),
            "lN": np.ascontiguousarray(local[sl]),
            "wT": wT,
            "wb": wb,
        })
    return in_maps


def _run(inputs, trace=False):
    from concourse.bass_utils import run_bass_kernel_spmd

    nc = _get_nc()
    in_maps = _prep_inputs(**inputs)
    res = run_bass_kernel_spmd(nc, in_maps, list(range(NCORES)), trace=trace)
    out = np.empty((B, T, D), dtype=np.float32)
    for c in range(NCORES):
        outT = res.results[c]["outT"]                    # [NB, d, t]
        out[c * NB:(c + 1) * NB] = outT.transpose(0, 2, 1)
    return out, res


def kernel(**inputs):
    out, _ = _run(inputs, trace=False)
    return out


# revision 22
# speedup vs baseline: 1.0344x; 1.0168x over previous
"""Trainium2 Bass kernel for visual cross-attention:
    proj   = text @ W_w.T + W_b          [B,T,D]
    scores = proj @ local.T              [B,T,L]
    attn   = softmax(scores, axis=-1)
    out    = attn @ local                [B,T,D]

B=16, T=L=D=1024, fp32. Data-parallel over batch: 8 cores x 2 batches.
All matmuls run as float32r (full PE rate, ~1.5e-4 rel err vs 4x-slower fp32).

Per core, per batch, per T-tile (512 t's):
  A: projT[e,t]   = W^T-chunks.T @ textT-chunks        (PE, accum over d)
  B: scores[t,l]  = projT-chunks.T @ localT-chunks     (PE, accum over e)
     softmax over l (free dim): DVE max, ACT exp(+bias,-max, accum sum),
     DVE reciprocal + row scale
  T: attnT[l,t]   = PE transpose of attn[t,l] 128x128 blocks
     (emitted one q-chunk behind B so the next B covers softmax latency)
  C: outT[d,t]    = local-chunks.T @ attnT-chunks      (PE, accum over l)
For batch 0 both T-tiles' A phases are emitted first: the startup is
DMA-bound (~14MB must land) and A is the only work whose data arrives first.
Host side only reshapes/transposes (layout prep + final [d,t]->[t,d]).
"""
import sys

sys.path.insert(0, "/opt/trn_rl_repo")
import numpy as np

B, T, L, D = 16, 1024, 1024, 1024
NCORES = 8
NB = B // NCORES          # batches per core
TT = 512                  # T-tile (moving dim for phases A/C)
NT = T // TT              # T-tiles per batch
NC8 = D // 128            # 128-chunks along d/e/l
NQ = TT // 128            # 128-t chunks per T-tile

_cache = {}


def _build():
    import concourse.tile as tile
    from concourse import bacc, mybir
    from concourse.masks import make_identity

    f32 = mybir.dt.float32
    f32r = mybir.dt.float32r
    Act = mybir.ActivationFunctionType

    nc = bacc.Bacc("TRN2", target_bir_lowering=False, debug=False,
                   num_devices=NCORES)
    tT_d = nc.dram_tensor("tT", [NB, D, T], f32r, kind="ExternalInput").ap()
    lT_d = nc.dram_tensor("lT", [NB, D, L], f32r, kind="ExternalInput").ap()
    lN_d = nc.dram_tensor("lN", [NB, L, D], f32r, kind="ExternalInput").ap()
    wT_d = nc.dram_tensor("wT", [D, D], f32r, kind="ExternalInput").ap()
    wb_d = nc.dram_tensor("wb", [128, NC8], f32, kind="ExternalInput").ap()
    outT_d = nc.dram_tensor("outT", [NB, D, T], f32, kind="ExternalOutput").ap()

    with tile.TileContext(nc) as tc:
        with tc.tile_pool(name="const", bufs=1) as constp, \
             tc.tile_pool(name="res", bufs=1) as resp, \
             tc.tile_pool(name="work", bufs=2) as workp, \
             tc.tile_pool(name="proj", bufs=2) as projp, \
             tc.tile_pool(name="single", bufs=1) as singlep, \
             tc.tile_pool(name="stats", bufs=8) as statsp, \
             tc.tile_pool(name="psS", bufs=2, space="PSUM") as psS_p, \
             tc.tile_pool(name="psMM", bufs=2, space="PSUM") as psMM_p, \
             tc.tile_pool(name="psT", bufs=2, space="PSUM") as psT_p:

            # round-robin loads across all 3 DMA-capable queues (sync/scalar
            # HWDGE + gpsimd SWDGE), in consumption order; each queue peaks
            # ~110-130GB/s, together ~350GB/s (HBM-bound).
            queues = [nc.sync, nc.scalar, nc.gpsimd]
            qi = [0]

            def load(out, in_):
                queues[qi[0] % 3].dma_start(out=out, in_=in_)
                qi[0] += 1

            def load_tT(b, it):
                t0 = it * TT
                tile_ = workp.tile([128, NC8, TT], f32r, tag="tT")
                for dc in range(NC8):
                    load(tile_[:, dc, :],
                         tT_d[b, dc * 128:(dc + 1) * 128, t0:t0 + TT])
                return tile_

            wt_sb = constp.tile([128, NC8, D], f32r, tag="wt")
            wb_sb = constp.tile([128, NC8], f32, tag="wb")
            tT_first = workp.tile([128, NC8, TT], f32r, tag="tT")
            for dc in range(NC8):
                load(wt_sb[:, dc, :], wT_d[dc * 128:(dc + 1) * 128, :])
                load(tT_first[:, dc, :], tT_d[0, dc * 128:(dc + 1) * 128, 0:TT])
            load(wb_sb[:], wb_d[:])
            identf = constp.tile([128, 128], f32, tag="identf")
            make_identity(nc, identf[:])
            ident = constp.tile([128, 128], f32r, tag="ident")
            nc.vector.tensor_copy(ident[:], identf[:])

            def phase_a(tT_sb):
                projT = projp.tile([128, NC8, TT], f32r, tag="projT")
                for ec in range(NC8):
                    psA = psMM_p.tile([128, TT], f32, tag="mm")
                    for dc in range(NC8):
                        nc.tensor.matmul(
                            psA[:],
                            wt_sb[:, dc, ec * 128:(ec + 1) * 128],
                            tT_sb[:, dc, :],
                            start=(dc == 0), stop=(dc == NC8 - 1))
                    nc.scalar.activation(projT[:, ec, :], psA[:], Act.Identity,
                                         bias=wb_sb[:, ec:ec + 1], scale=1.0)
                return projT

            def transposes(attnT, et, q):
                for lq in range(NC8):
                    psT = psT_p.tile([128, 128], f32r, tag="tp")
                    nc.tensor.transpose(psT[:], et[:, lq * 128:(lq + 1) * 128],
                                        ident[:])
                    dst = attnT[:, lq, q * 128:(q + 1) * 128]
                    if lq % 2 == 0:
                        nc.vector.tensor_copy(dst, psT[:])
                    else:
                        nc.scalar.copy(dst, psT[:])

            # ---- batch-0 startup choreography ----
            # The first exp needs wt+tT(0,0)+tT(0,1)+tT(1,0)+lT(0) = 14MB of
            # HBM; PE bridge work (3 A phases + B-lo) is emitted first, in
            # the DMA arrival order, so the PE never goes HAM-cold.
            tT_b00 = tT_first

            tT_b01 = load_tT(0, 1)
            lT_b0 = resp.tile([128, NC8, L], f32r, tag="lT")
            lN_b0 = resp.tile([128, NC8, D], f32r, tag="lN")
            for c in range(NC8):
                load(lT_b0[:, c, 0:512], lT_d[0, c * 128:(c + 1) * 128, 0:512])
            for c in range(NC8):
                load(lT_b0[:, c, 512:L], lT_d[0, c * 128:(c + 1) * 128, 512:L])
            for c in range(NC8):
                load(lN_b0[:, c, :], lN_d[0, c * 128:(c + 1) * 128, :])
            pre_proj = {(0, 0): phase_a(tT_b00), (0, 1): phase_a(tT_b01)}

            for b in range(NB):
                if b == 0:
                    lT_sb, lN_sb = lT_b0, lN_b0
                else:
                    lT_sb = resp.tile([128, NC8, L], f32r, tag="lT")
                    lN_sb = resp.tile([128, NC8, D], f32r, tag="lN")
                    for c in range(NC8):
                        load(lT_sb[:, c, :], lT_d[b, c * 128:(c + 1) * 128, :])
                    for c in range(NC8):
                        load(lN_sb[:, c, :], lN_d[b, c * 128:(c + 1) * 128, :])

                projTs = [pre_proj.get((b, it)) for it in range(NT)]
                tT_tiles = [None] * NT

                for it in range(NT):
                    t0 = it * TT
                    if projTs[it] is None:
                        if tT_tiles[it] is None:
                            tT_tiles[it] = load_tT(b, it)
                        projTs[it] = phase_a(tT_tiles[it])
                    projT = projTs[it]
                    # ---- phase B + softmax, transposes one q behind ----
                    attnT = singlep.tile([128, NC8, TT], f32r, tag="attnT")
                    pending = None
                    for q in range(NQ):
                        psS = psS_p.tile([128, L], f32, tag="scores")
                        for lh in range(L // 512):
                            for ec in range(NC8):
                                nc.tensor.matmul(
                                    psS[:, lh * 512:(lh + 1) * 512],
                                    projT[:, ec, q * 128:(q + 1) * 128],
                                    lT_sb[:, ec, lh * 512:(lh + 1) * 512],
                                    start=(ec == 0), stop=(ec == NC8 - 1))
                        nm = statsp.tile([128, 1], f32, tag="nm")
                        nc.vector.tensor_reduce(nm[:], psS[:],
                                                axis=mybir.AxisListType.X,
                                                op=mybir.AluOpType.max,
                                                negate=True)
                        et = workp.tile([128, L], f32r, tag="et")
                        s = statsp.tile([128, 1], f32, tag="s")
                        nc.scalar.activation(et[:], psS[:], Act.Exp,
                                             bias=nm[:, 0:1], scale=1.0,
                                             accum_out=s[:])
                        rr = statsp.tile([128, 1], f32, tag="rr")
                        nc.vector.reciprocal(rr[:], s[:])
                        nc.vector.tensor_scalar_mul(et[:], et[:], rr[:, 0:1])
                        if pending is not None:
                            transposes(attnT, *pending)
                        pending = (et, q)
                    transposes(attnT, *pending)
                    # ---- phase C: outT[d, t] ----
                    for dc in range(NC8):
                        psC = psMM_p.tile([128, TT], f32, tag="mm")
                        for lq in range(NC8):
                            nc.tensor.matmul(
                                psC[:],
                                lN_sb[:, lq, dc * 128:(dc + 1) * 128],
                                attnT[:, lq, :],
                                start=(lq == 0), stop=(lq == NC8 - 1))
                        outcp = workp.tile([128, TT], f32, tag="outcp")
                        nc.scalar.copy(outcp[:], psC[:])
                        queues[dc % 3].dma_start(
                            out=outT_d[b, dc * 128:(dc + 1) * 128, t0:t0 + TT],
                            in_=outcp[:])
    nc.compile()
    return nc


def _get_nc():
    if "nc" not in _cache:
        _cache["nc"] = _build()
    return _cache["nc"]


def _prep_inputs(text_features, local_features, W_w, W_b):
    text = np.asarray(text_features, dtype=np.float32)
    local = np.asarray(local_features, dtype=np.float32)
    W = np.asarray(W_w, dtype=np.float32)
    bvec = np.asarray(W_b, dtype=np.float32)

    wT = np.ascontiguousarray(W.T)                       # [d, e]
    wb = np.ascontiguousarray(bvec.reshape(NC8, 128).T)  # [128, ec]
    in_maps = []
    for c in range(NCORES):
        sl = slice(c * NB, (c + 1) * NB)
        in_maps.append({
            "tT": np.ascontiguousarray(text[sl].transpose(0, 2, 1)),
            "lT": np.ascontiguousarray(local[sl].transpose(0, 2, 1)),
            "lN": np.ascontiguousarray(local[sl]),
            "wT": wT,
            "wb": wb,
        })
    return in_maps


def _run(inputs, trace=False):
    from concourse.bass_utils import run_bass_kernel_spmd

    nc = _get_nc()
    in_maps = _prep_inputs(**inputs)
    res = run_bass_kernel_spmd(nc, in_maps, list(range(NCORES)), trace=trace)
    out = np.empty((B, T, D), dtype=np.float32)
    for c in range(NCORES):
        outT = res.results[c]["outT"]                    # [NB, d, t]
        out[c * NB:(c + 1) * NB] = outT.transpose(0, 2, 1)
    return out, res


def kernel(**inputs):
    out, _ = _run(inputs, trace=False)
    return out


# revision 24
# speedup vs baseline: 1.0852x; 1.0491x over previous
"""Trainium2 Bass kernel for visual cross-attention:
    proj   = text @ W_w.T + W_b          [B,T,D]
    scores = proj @ local.T              [B,T,L]
    attn   = softmax(scores, axis=-1)
    out    = attn @ local                [B,T,D]

B=16, T=L=D=1024, fp32. Data-parallel over batch: 8 cores x 2 batches.
All matmuls run as float32r (full PE rate, ~1.5e-4 rel err vs 4x-slower fp32).

Per core, per batch, per T-tile (512 t's):
  A: projT[e,t]   = W^T-chunks.T @ textT-chunks        (PE, accum over d)
  B: scores[t,l]  = projT-chunks.T @ localT-chunks     (PE, accum over e)
     softmax over l (free dim): DVE max, ACT exp(+bias,-max, accum sum),
     DVE reciprocal + row scale
  T: attnT[l,t]   = PE transpose of attn[t,l] 128x128 blocks
     (emitted one q-chunk behind B so the next B covers softmax latency)
  C: outT[d,t]    = local-chunks.T @ attnT-chunks      (PE, accum over l)
For batch 0 both T-tiles' A phases are emitted first: the startup is
DMA-bound (~14MB must land) and A is the only work whose data arrives first.
Host side only reshapes/transposes (layout prep + final [d,t]->[t,d]).
"""
import sys

sys.path.insert(0, "/opt/trn_rl_repo")
import numpy as np

B, T, L, D = 16, 1024, 1024, 1024
NCORES = 8
NB = B // NCORES          # batches per core
TT = 512                  # T-tile (moving dim for phases A/C)
NT = T // TT              # T-tiles per batch
NC8 = D // 128            # 128-chunks along d/e/l
NQ = TT // 128            # 128-t chunks per T-tile

_cache = {}


def _build():
    import concourse.tile as tile
    from concourse import bacc, mybir
    from concourse.masks import make_identity

    f32 = mybir.dt.float32
    f32r = mybir.dt.float32r
    Act = mybir.ActivationFunctionType

    nc = bacc.Bacc("TRN2", target_bir_lowering=False, debug=False,
                   num_devices=NCORES)
    tT_d = nc.dram_tensor("tT", [NB, D, T], f32r, kind="ExternalInput").ap()
    lT_d = nc.dram_tensor("lT", [NB, D, L], f32r, kind="ExternalInput").ap()
    lN_d = nc.dram_tensor("lN", [NB, L, D], f32r, kind="ExternalInput").ap()
    wT_d = nc.dram_tensor("wT", [D, D], f32r, kind="ExternalInput").ap()
    wb_d = nc.dram_tensor("wb", [128, NC8], f32, kind="ExternalInput").ap()
    outT_d = nc.dram_tensor("outT", [NB, D, T], f32, kind="ExternalOutput").ap()

    with tile.TileContext(nc) as tc:
        with tc.tile_pool(name="const", bufs=1) as constp, \
             tc.tile_pool(name="res", bufs=1) as resp, \
             tc.tile_pool(name="work", bufs=2) as workp, \
             tc.tile_pool(name="proj", bufs=3) as projp, \
             tc.tile_pool(name="single", bufs=1) as singlep, \
             tc.tile_pool(name="stats", bufs=8) as statsp, \
             tc.tile_pool(name="psS", bufs=2, space="PSUM") as psS_p, \
             tc.tile_pool(name="psMM", bufs=2, space="PSUM") as psMM_p, \
             tc.tile_pool(name="psT", bufs=2, space="PSUM") as psT_p:

            # round-robin loads across all 3 DMA-capable queues (sync/scalar
            # HWDGE + gpsimd SWDGE), in consumption order; each queue peaks
            # ~110-130GB/s, together ~350GB/s (HBM-bound).
            queues = [nc.sync, nc.scalar, nc.gpsimd]
            qi = [0]

            def load(out, in_):
                queues[qi[0] % 3].dma_start(out=out, in_=in_)
                qi[0] += 1

            def load_tT(b, it):
                t0 = it * TT
                tile_ = workp.tile([128, NC8, TT], f32r, tag="tT")
                for dc in range(NC8):
                    load(tile_[:, dc, :],
                         tT_d[b, dc * 128:(dc + 1) * 128, t0:t0 + TT])
                return tile_

            wt_sb = constp.tile([128, NC8, D], f32r, tag="wt")
            wb_sb = constp.tile([128, NC8], f32, tag="wb")
            tT_first = workp.tile([128, NC8, TT], f32r, tag="tT")
            for dc in range(NC8):
                load(wt_sb[:, dc, :], wT_d[dc * 128:(dc + 1) * 128, :])
                load(tT_first[:, dc, :], tT_d[0, dc * 128:(dc + 1) * 128, 0:TT])
            load(wb_sb[:], wb_d[:])
            identf = constp.tile([128, 128], f32, tag="identf")
            make_identity(nc, identf[:])
            ident = constp.tile([128, 128], f32r, tag="ident")
            nc.vector.tensor_copy(ident[:], identf[:])

            def phase_a(tT_sb):
                projT = projp.tile([128, NC8, TT], f32r, tag="projT")
                for ec in range(NC8):
                    psA = psMM_p.tile([128, TT], f32, tag="mm")
                    for dc in range(NC8):
                        nc.tensor.matmul(
                            psA[:],
                            wt_sb[:, dc, ec * 128:(ec + 1) * 128],
                            tT_sb[:, dc, :],
                            start=(dc == 0), stop=(dc == NC8 - 1))
                    nc.scalar.activation(projT[:, ec, :], psA[:], Act.Identity,
                                         bias=wb_sb[:, ec:ec + 1], scale=1.0)
                return projT

            def transposes(attnT, et, q):
                for lq in range(NC8):
                    psT = psT_p.tile([128, 128], f32r, tag="tp")
                    nc.tensor.transpose(psT[:], et[:, lq * 128:(lq + 1) * 128],
                                        ident[:])
                    dst = attnT[:, lq, q * 128:(q + 1) * 128]
                    if lq % 2 == 0:
                        nc.vector.tensor_copy(dst, psT[:])
                    else:
                        nc.scalar.copy(dst, psT[:])

            # ---- batch-0 startup choreography ----
            # The first exp needs wt+tT(0,0)+tT(0,1)+tT(1,0)+lT(0) = 14MB of
            # HBM; PE bridge work (3 A phases + B-lo) is emitted first, in
            # the DMA arrival order, so the PE never goes HAM-cold.
            tT_b00 = tT_first

            tT_b01 = load_tT(0, 1)
            lT_b0 = resp.tile([128, NC8, L], f32r, tag="lT")
            lN_b0 = resp.tile([128, NC8, D], f32r, tag="lN")
            for c in range(NC8):
                load(lT_b0[:, c, 0:512], lT_d[0, c * 128:(c + 1) * 128, 0:512])
            for c in range(NC8):
                load(lT_b0[:, c, 512:L], lT_d[0, c * 128:(c + 1) * 128, 512:L])
            for c in range(NC8):
                load(lN_b0[:, c, :], lN_d[0, c * 128:(c + 1) * 128, :])
            projTs = {(0, 0): phase_a(tT_b00), (0, 1): phase_a(tT_b01)}
            lT_tiles = {0: lT_b0}
            lN_tiles = {0: lN_b0}

            def load_locals(b):
                lT_sb = resp.tile([128, NC8, L], f32r, tag="lT")
                lN_sb = resp.tile([128, NC8, D], f32r, tag="lN")
                for c in range(NC8):
                    load(lT_sb[:, c, :], lT_d[b, c * 128:(c + 1) * 128, :])
                for c in range(NC8):
                    load(lN_sb[:, c, :], lN_d[b, c * 128:(c + 1) * 128, :])
                lT_tiles[b] = lT_sb
                lN_tiles[b] = lN_sb

            tiles = [(b, it) for b in range(NB) for it in range(NT)]
            for i, (b, it) in enumerate(tiles):
                t0 = it * TT
                if b > 0 and it == 0:
                    load_locals(b)
                projT = projTs[(b, it)]
                lT_sb, lN_sb = lT_tiles[b], lN_tiles[b]
                # ---- phase B + softmax, transposes one q behind ----
                attnT = singlep.tile([128, NC8, TT], f32r, tag="attnT")
                pending = None
                for q in range(NQ):
                    psS = psS_p.tile([128, L], f32, tag="scores")
                    for lh in range(L // 512):
                        for ec in range(NC8):
                            nc.tensor.matmul(
                                psS[:, lh * 512:(lh + 1) * 512],
                                projT[:, ec, q * 128:(q + 1) * 128],
                                lT_sb[:, ec, lh * 512:(lh + 1) * 512],
                                start=(ec == 0), stop=(ec == NC8 - 1))
                    nm = statsp.tile([128, 1], f32, tag="nm")
                    nc.vector.tensor_reduce(nm[:], psS[:],
                                            axis=mybir.AxisListType.X,
                                            op=mybir.AluOpType.max,
                                            negate=True)
                    et = workp.tile([128, L], f32r, tag="et")
                    s = statsp.tile([128, 1], f32, tag="s")
                    nc.scalar.activation(et[:], psS[:], Act.Exp,
                                         bias=nm[:, 0:1], scale=1.0,
                                         accum_out=s[:])
                    rr = statsp.tile([128, 1], f32, tag="rr")
                    nc.vector.reciprocal(rr[:], s[:])
                    nc.vector.tensor_scalar_mul(et[:], et[:], rr[:, 0:1])
                    if pending is not None:
                        transposes(attnT, *pending)
                    pending = (et, q)
                # prefetch the next tile's A phase here: its matmuls fill
                # the exp(q3)->transpose latency bubble and the batch
                # boundary, instead of the PE idling on them
                if i + 1 < len(tiles):
                    nb_, nit_ = tiles[i + 1]
                    if (nb_, nit_) not in projTs:
                        projTs[(nb_, nit_)] = phase_a(load_tT(nb_, nit_))
                transposes(attnT, *pending)
                # ---- phase C: outT[d, t] ----
                for dc in range(NC8):
                    psC = psMM_p.tile([128, TT], f32, tag="mm")
                    for lq in range(NC8):
                        nc.tensor.matmul(
                            psC[:],
                            lN_sb[:, lq, dc * 128:(dc + 1) * 128],
                            attnT[:, lq, :],
                            start=(lq == 0), stop=(lq == NC8 - 1))
                    outcp = workp.tile([128, TT], f32, tag="outcp")
                    nc.scalar.copy(outcp[:], psC[:])
                    queues[dc % 3].dma_start(
                        out=outT_d[b, dc * 128:(dc + 1) * 128, t0:t0 + TT],
                        in_=outcp[:])
    nc.compile()
    return nc


def _get_nc():
    if "nc" not in _cache:
        _cache["nc"] = _build()
    return _cache["nc"]


def _prep_inputs(text_features, local_features, W_w, W_b):
    text = np.asarray(text_features, dtype=np.float32)
    local = np.asarray(local_features, dtype=np.float32)
    W = np.asarray(W_w, dtype=np.float32)
    bvec = np.asarray(W_b, dtype=np.float32)

    wT = np.ascontiguousarray(W.T)                       # [d, e]
    wb = np.ascontiguousarray(bvec.reshape(NC8, 128).T)  # [128, ec]
    in_maps = []
    for c in range(NCORES):
        sl = slice(c * NB, (c + 1) * NB)
        in_maps.append({
            "tT": np.ascontiguousarray(text[sl].transpose(0, 2, 1)),
            "lT": np.ascontiguousarray(local[sl].transpose(0, 2, 1)),
            "lN": np.ascontiguousarray(local[sl]),
            "wT": wT,
            "wb": wb,
        })
    return in_maps


def _run(inputs, trace=False):
    from concourse.bass_utils import run_bass_kernel_spmd

    nc = _get_nc()
    in_maps = _prep_inputs(**inputs)
    res = run_bass_kernel_spmd(nc, in_maps, list(range(NCORES)), trace=trace)
    out = np.empty((B, T, D), dtype=np.float32)
    for c in range(NCORES):
        outT = res.results[c]["outT"]                    # [NB, d, t]
        out[c * NB:(c + 1) * NB] = outT.transpose(0, 2, 1)
    return out, res


def kernel(**inputs):
    out, _ = _run(inputs, trace=False)
    return out
